# revision 26
# baseline (speedup 1.0000x reference)
"""AdditiveAttention (Bahdanau) kernel for Trainium2, 8-way data-parallel over batch.

Math (per batch row b):
    q = h_t @ w_h.T                       [ATTN]
    k[t] = memory[t] @ w_m.T              [T, ATTN]
    e[t] = v . tanh(q + k[t])             [T]
    attn = softmax(mask ? e : -inf)       [T]
    context = attn @ memory               [MEM]

Strategy: the boolean mask kills ~half the rows exactly (attn == 0 there), so
the device only ever touches the unmasked rows.  The host computes, per batch,
the sorted list of kept row indices (padded to NKEEP with index 0 and an
additive -1e30 bias so padded slots vanish in the softmax), and the kernel
gathers exactly those rows with dma_gather.  Everything else is a single
fused pass per batch: gather -> bf16 cast -> PE transpose (for the k matmul,
which contracts over MEM) -> k matmul -> tanh(+q) -> e matvec -> masked
softmax -> attn transpose -> context matmul.  Context/attn are scattered back
to dense [B, T] on the host.
"""

import math
from dataclasses import dataclass

import numpy as np


@dataclass(frozen=True)
class Cfg:
    BL: int = 8  # batches per core
    T: int = 4096
    MEM: int = 1024
    HID: int = 1024
    ATTN: int = 256
    NKEEP: int = 2304  # 18*128; P(Binom(4096,.5) > 2304) ~ 6e-16
    GCH: int = 768  # gather chunk (rows per dma_gather), multiple of 128

    @property
    def MC(self):
        return self.MEM // 128

    @property
    def AC(self):
        return self.ATTN // 128

    @property
    def HC(self):
        return self.HID // 128

    @property
    def TSEG(self):
        return self.GCH // 2  # matmul moving-dim segment, <=512

    @property
    def NCH(self):
        return self.NKEEP // self.GCH  # gather chunks per batch

    @property
    def TC(self):
        return self.NKEEP // 128  # 128-row chunks per batch


FULL = Cfg()


def build(cfg: Cfg):
    import concourse.mybir as mybir
    import concourse.tile as tile
    from concourse import bacc

    f32 = mybir.dt.float32
    bf16 = mybir.dt.bfloat16
    i16 = mybir.dt.int16
    Act = mybir.ActivationFunctionType

    BL, MEM, HID, ATTN, NKEEP, GCH = (
        cfg.BL,
        cfg.MEM,
        cfg.HID,
        cfg.ATTN,
        cfg.NKEEP,
        cfg.GCH,
    )
    MC, AC, HC, TSEG, NCH, TC = cfg.MC, cfg.AC, cfg.HC, cfg.TSEG, cfg.NCH, cfg.TC
    ICOL = NKEEP // 16  # idx columns total
    ICH = GCH // 16  # idx columns per gather chunk
    GTC = GCH // 128  # 128-row chunks per gather chunk

    nc = bacc.Bacc("TRN2", target_bir_lowering=False, debug=False)

    mem_d = nc.dram_tensor("mem", [BL, cfg.T, MEM], f32, kind="ExternalInput")
    hT_d = nc.dram_tensor("hT", [HID, BL], f32, kind="ExternalInput")
    whT_d = nc.dram_tensor("whT", [HID, ATTN], f32, kind="ExternalInput")
    wmT_d = nc.dram_tensor("wmT", [MEM, ATTN], bf16, kind="ExternalInput")
    vT_d = nc.dram_tensor("vT", [ATTN, 1], bf16, kind="ExternalInput")
    idx_d = nc.dram_tensor("idx", [128, BL, ICOL], i16, kind="ExternalInput")
    id_d = nc.dram_tensor("idm", [128, 128], bf16, kind="ExternalInput")
    kb_d = nc.dram_tensor("kb", [BL, NKEEP], f32, kind="ExternalInput")
    attn_o = nc.dram_tensor("attn_o", [BL, NKEEP], f32, kind="ExternalOutput")
    ctx_o = nc.dram_tensor("ctx_o", [BL, MEM], f32, kind="ExternalOutput")

    with tile.TileContext(nc) as tc:
        with tc.tile_pool(name="const", bufs=1) as const:
            wm_sb = const.tile([128, MC, ATTN], bf16)
            nc.sync.dma_start(wm_sb[:], wmT_d.rearrange("(mc p) a -> p mc a", p=128))
            vT_sb = const.tile([128, AC, 1], bf16)
            nc.sync.dma_start(vT_sb[:], vT_d.rearrange("(ac p) one -> p ac one", p=128))
            ident = const.tile([128, 128], bf16)
            nc.sync.dma_start(ident[:], id_d[:])
            ident1 = const.tile([1, 1], f32)
            nc.vector.memset(ident1[:], 1.0)
            idx_sb = const.tile([128, BL, ICOL], i16)
            nc.sync.dma_start(idx_sb[:], idx_d[:])
            q_sb = const.tile([128, AC, BL], f32)

            # ---- q = (w_h @ h) computed as q_T[a, b], contraction over HID
            with (
                tc.tile_pool(name="setup", bufs=1) as setup,
                tc.tile_pool(name="psum_q", bufs=1, space="PSUM") as psum_q,
            ):
                whT_sb = setup.tile([128, HC, ATTN], f32)
                nc.sync.dma_start(
                    whT_sb[:], whT_d.rearrange("(hc p) a -> p hc a", p=128)
                )
                hT_sb = setup.tile([128, HC, BL], f32)
                nc.sync.dma_start(hT_sb[:], hT_d.rearrange("(hc p) b -> p hc b", p=128))
                for a in range(AC):
                    q_ps = psum_q.tile([128, BL], f32)
                    for hc in range(HC):
                        nc.tensor.matmul(
                            q_ps[:],
                            whT_sb[:, hc, a * 128 : (a + 1) * 128],
                            hT_sb[:, hc, :],
                            start=(hc == 0),
                            stop=(hc == HC - 1),
                        )
                    nc.vector.tensor_copy(q_sb[:, a, :], q_ps[:])

            with (
                tc.tile_pool(name="g32", bufs=2) as g32p,
                tc.tile_pool(name="gbf", bufs=3) as gbfp,
                tc.tile_pool(name="mtp", bufs=2) as mtpp,
                tc.tile_pool(name="th", bufs=3) as thp,
                tc.tile_pool(name="sm", bufs=2) as smp,
                tc.tile_pool(name="esm", bufs=1) as esmp,
                tc.tile_pool(name="eseg", bufs=3) as esegp,
                tc.tile_pool(name="out", bufs=2) as outp,
                tc.tile_pool(name="ps_t", bufs=2, space="PSUM") as ps_t,
                tc.tile_pool(name="ps_k", bufs=2, space="PSUM") as ps_k,
                tc.tile_pool(name="ps_e", bufs=1, space="PSUM") as ps_e,
                tc.tile_pool(name="ps_a", bufs=1, space="PSUM") as ps_a,
                tc.tile_pool(name="ps_c", bufs=1, space="PSUM") as ps_c,
            ):
                # Softmax note: |e| <= ||v||_1 ~ 8, so exp() cannot overflow
                # and the classic max-subtraction is skipped entirely.  That
                # removes the all-of-e barrier: exp/transposes run per segment.
                CSEG = min(512, MEM)
                NMH = MEM // CSEG
                # uneven gather chunks: 1024-row chunks + remainder, so the
                # k matmuls run at the full 512-column moving dim
                chunks = []
                off = 0
                while off < NKEEP:
                    n = min(1024, NKEEP - off)
                    chunks.append((off, n))
                    off += n
                nseg_total = sum(
                    (n + 511) // 512 for _, n in chunks
                )
                for b in range(BL):
                    kb_t = esmp.tile([1, NKEEP], f32, tag="kb")
                    nc.sync.dma_start(kb_t[:], kb_d[b : b + 1, :])
                    p_sb = smp.tile([1, NKEEP], f32, tag="p")
                    sc = smp.tile([1, 64], f32, tag="scalars")
                    ds = sc[:, 0:nseg_total]  # per-seg exp sums
                    rd = sc[:, 32:33]
                    atps = ps_a.tile([128, TC], f32, tag="atps")
                    at_bf = outp.tile([128, TC], bf16, tag="atbf")
                    cps = ps_c.tile([1, MEM], f32, tag="cps")

                    ev = 0
                    seg_i = 0
                    for co, cn in chunks:
                        GTCc = cn // 128
                        segs = [
                            (so, min(512, cn - so)) for so in range(0, cn, 512)
                        ]
                        g32 = g32p.tile([128, GTCc, MEM], f32, tag="g32")
                        nc.gpsimd.dma_gather(
                            g32[:],
                            mem_d[b],
                            idx_sb[:, b, co // 16 : (co + cn) // 16],
                            num_idxs=cn,
                            num_idxs_reg=cn,
                            elem_size=MEM,
                        )
                        # cast to bf16, split fine for pipelining
                        gbf = gbfp.tile([128, GTCc, MEM], bf16, tag="gbf")
                        for g in range(GTCc):
                            nc.vector.tensor_copy(gbf[:, g, :], g32[:, g, :])
                        # transpose this chunk: [t', m] -> [m, t'] per 128-block
                        mtp = mtpp.tile([128, MC, cn], bf16, tag="mtp")
                        for mc in range(MC):
                            tps = ps_t.tile([128, cn], bf16, tag="tps")
                            for g in range(GTCc):
                                nc.tensor.transpose(
                                    tps[:, g * 128 : (g + 1) * 128],
                                    gbf[:, g, mc * 128 : (mc + 1) * 128],
                                    ident[:],
                                )
                            if ev % 4 != 3:
                                nc.vector.tensor_copy(mtp[:, mc, :], tps[:])
                            else:
                                nc.scalar.copy(mtp[:, mc, :], tps[:])
                            ev += 1
                        # k matmuls + tanh per (a-chunk, segment)
                        ths = []
                        for a in range(AC):
                            th_t = thp.tile([128, cn], bf16, tag="th")
                            ths.append(th_t)
                            for so, sn in segs:
                                kps = ps_k.tile([128, 512], f32, tag="kps")
                                for mc in range(MC):
                                    nc.tensor.matmul(
                                        kps[:, :sn],
                                        wm_sb[:, mc, a * 128 : (a + 1) * 128],
                                        mtp[:, mc, so : so + sn],
                                        start=(mc == 0),
                                        stop=(mc == MC - 1),
                                    )
                                nc.scalar.activation(
                                    out=th_t[:, so : so + sn],
                                    in_=kps[:, :sn],
                                    func=Act.Tanh,
                                    bias=q_sb[:, a, b : b + 1],
                                    scale=1.0,
                                )
                        # e matvec per segment; pad-bias folded into psum evac;
                        # exp runs per segment (no max barrier)
                        for so, sn in segs:
                            off2 = co + so
                            eps = ps_e.tile([1, 512], f32, tag="eps")
                            for a in range(AC):
                                nc.tensor.matmul(
                                    eps[:, :sn],
                                    vT_sb[:, a, :],
                                    ths[a][:, so : so + sn],
                                    start=(a == 0),
                                    stop=(a == AC - 1),
                                )
                            e_seg = esegp.tile([1, 512], f32, tag="eseg")
                            nc.vector.tensor_add(
                                e_seg[:, :sn], eps[:, :sn], kb_t[:, off2 : off2 + sn]
                            )
                            nc.scalar.activation(
                                out=p_sb[:, off2 : off2 + sn],
                                in_=e_seg[:, :sn],
                                func=Act.Exp,
                                accum_out=ds[:, seg_i : seg_i + 1],
                            )
                            seg_i += 1
                        # transpose p for this chunk and accumulate context
                        for g in range(GTCc):
                            tcc = co // 128 + g
                            nc.tensor.transpose(
                                atps[:, tcc : tcc + 1],
                                p_sb[:, tcc * 128 : (tcc + 1) * 128],
                                ident1[:],
                            )
                        nc.vector.tensor_copy(
                            at_bf[:, co // 128 : co // 128 + GTCc],
                            atps[:, co // 128 : co // 128 + GTCc],
                        )
                        for mh in range(NMH):
                            for g in range(GTCc):
                                tcc = co // 128 + g
                                nc.tensor.matmul(
                                    cps[:, mh * CSEG : (mh + 1) * CSEG],
                                    at_bf[:, tcc : tcc + 1],
                                    gbf[:, g, mh * CSEG : (mh + 1) * CSEG],
                                    start=(tcc == 0),
                                    stop=(tcc == TC - 1),
                                )

                    # ---- denominator + normalization (context scaled by 1/D
                    # during psum evac; attn normalized on the way out)
                    nc.vector.reduce_sum(rd, ds, axis=mybir.AxisListType.X)
                    nc.vector.reciprocal(rd, rd)
                    attn_n = esmp.tile([1, NKEEP], f32, tag="attn_n")
                    nc.scalar.activation(
                        out=attn_n[:], in_=p_sb[:], func=Act.Copy, scale=rd
                    )
                    nc.sync.dma_start(attn_o[b : b + 1, :], attn_n[:])
                    ctx_sb = outp.tile([1, MEM], f32, tag="ctx")
                    nc.vector.tensor_scalar_mul(ctx_sb[:], cps[:], rd)
                    nc.sync.dma_start(ctx_o[b : b + 1, :], ctx_sb[:])

    nc.compile()
    return nc


def host_prep(cfg: Cfg, memory_mask_local: np.ndarray):
    """Per-core index/bias prep. memory_mask_local: [BL, T] bool.

    Returns idx [128, BL, NKEEP//16] i16, kb [BL, NKEEP] f32, and the raw
    per-batch kept index lists (for host-side scatter of outputs).
    """
    BL, NKEEP = cfg.BL, cfg.NKEEP
    idx = np.zeros((128, BL, NKEEP // 16), dtype=np.int16)
    kb = np.zeros((BL, NKEEP), dtype=np.float32)
    kept_lists = []
    for b in range(BL):
        kept = np.nonzero(memory_mask_local[b])[0]
        cnt = len(kept)
        assert cnt <= NKEEP, f"kept count {cnt} exceeds NKEEP={NKEEP}"
        kept_lists.append(kept)
        arr = np.zeros(NKEEP, dtype=np.int16)
        arr[:cnt] = kept.astype(np.int16)
        kb[b, cnt:] = -1e30
        wrapped = arr.reshape(NKEEP // 16, 16).T  # [16, cols]
        idx[:, b, :] = np.tile(wrapped, (8, 1))
    return idx, kb, kept_lists


_CACHE = {}

# Test knobs (harness-invisible): TRACE captures an NTFF profile on the run;
# LAST_RESULT holds the most recent BassKernelResults.
TRACE = False
LAST_RESULT = None


def _get_module():
    if "nc" not in _CACHE:
        _CACHE["nc"] = build(FULL)
    return _CACHE["nc"]


def kernel(h_t, memory, memory_mask, w_h, w_m, v):
    import ml_dtypes

    cfg = FULL
    B, T, MEM = memory.shape
    n_cores = 8
    BL = B // n_cores
    assert BL == cfg.BL and T == cfg.T and MEM == cfg.MEM

    h_t = np.asarray(h_t, dtype=np.float32)
    memory = np.asarray(memory, dtype=np.float32)
    memory_mask = np.asarray(memory_mask)
    w_h = np.asarray(w_h, dtype=np.float32)
    w_m = np.asarray(w_m, dtype=np.float32)
    v = np.asarray(v, dtype=np.float32)

    whT = np.ascontiguousarray(w_h.T)  # [HID, ATTN] f32
    wmT = np.ascontiguousarray(w_m.T).astype(ml_dtypes.bfloat16)  # [MEM, ATTN]
    vT = np.ascontiguousarray(v.reshape(1, -1).T).astype(ml_dtypes.bfloat16)

    idm = np.eye(128, dtype=ml_dtypes.bfloat16)
    in_maps = []
    kept_all = []
    for c in range(n_cores):
        sl = slice(c * BL, (c + 1) * BL)
        idx, kb, kept_lists = host_prep(cfg, memory_mask[sl])
        kept_all.append(kept_lists)
        in_maps.append(
            {
                "mem": np.ascontiguousarray(memory[sl]),
                "hT": np.ascontiguousarray(h_t[sl].T),
                "whT": whT,
                "wmT": wmT,
                "vT": vT,
                "idx": idx,
                "kb": kb,
                "idm": idm,
            }
        )

    from concourse.bass_utils import run_bass_kernel_spmd

    nc = _get_module()
    res = run_bass_kernel_spmd(nc, in_maps, core_ids=list(range(n_cores)), trace=TRACE)
    globals()["LAST_RESULT"] = res

    context = np.zeros((B, MEM), dtype=np.float32)
    attn = np.zeros((B, T), dtype=np.float32)
    for c in range(n_cores):
        r = res.results[c]
        for b in range(cfg.BL):
            g = c * cfg.BL + b
            kept = kept_all[c][b]
            attn[g, kept] = r["attn_o"][b, : len(kept)]
            context[g] = r["ctx_o"][b]
    return context, attn


# revision 27
# speedup vs baseline: 1.1312x; 1.1312x over previous
"""AdditiveAttention (Bahdanau) kernel for Trainium2, 8-way data-parallel over batch.

Math (per batch row b):
    q = h_t @ w_h.T                       [ATTN]
    k[t] = memory[t] @ w_m.T              [T, ATTN]
    e[t] = v . tanh(q + k[t])             [T]
    attn = softmax(mask ? e : -inf)       [T]
    context = attn @ memory               [MEM]

Strategy: the boolean mask kills ~half the rows exactly (attn == 0 there), so
the device only ever touches the unmasked rows.  The host computes, per batch,
the sorted list of kept row indices (padded to NKEEP with index 0 and an
additive -1e30 bias so padded slots vanish in the softmax), and the kernel
gathers exactly those rows with dma_gather.  Everything else is a single
fused pass per batch: gather -> bf16 cast -> PE transpose (for the k matmul,
which contracts over MEM) -> k matmul -> tanh(+q) -> e matvec -> masked
softmax -> attn transpose -> context matmul.  Context/attn are scattered back
to dense [B, T] on the host.
"""

import math
from dataclasses import dataclass

import numpy as np


@dataclass(frozen=True)
class Cfg:
    BL: int = 8  # batches per core
    T: int = 4096
    MEM: int = 1024
    HID: int = 1024
    ATTN: int = 256
    NKEEP: int = 2304  # 18*128; P(Binom(4096,.5) > 2304) ~ 6e-16
    GCH: int = 768  # gather chunk (rows per dma_gather), multiple of 128

    @property
    def MC(self):
        return self.MEM // 128

    @property
    def AC(self):
        return self.ATTN // 128

    @property
    def HC(self):
        return self.HID // 128

    @property
    def TSEG(self):
        return self.GCH // 2  # matmul moving-dim segment, <=512

    @property
    def NCH(self):
        return self.NKEEP // self.GCH  # gather chunks per batch

    @property
    def TC(self):
        return self.NKEEP // 128  # 128-row chunks per batch


FULL = Cfg()


def build(cfg: Cfg):
    import concourse.mybir as mybir
    import concourse.tile as tile
    from concourse import bacc

    f32 = mybir.dt.float32
    bf16 = mybir.dt.bfloat16
    i16 = mybir.dt.int16
    Act = mybir.ActivationFunctionType

    BL, MEM, HID, ATTN, NKEEP, GCH = (
        cfg.BL,
        cfg.MEM,
        cfg.HID,
        cfg.ATTN,
        cfg.NKEEP,
        cfg.GCH,
    )
    MC, AC, HC, TSEG, NCH, TC = cfg.MC, cfg.AC, cfg.HC, cfg.TSEG, cfg.NCH, cfg.TC
    ICOL = NKEEP // 16  # idx columns total
    ICH = GCH // 16  # idx columns per gather chunk
    GTC = GCH // 128  # 128-row chunks per gather chunk

    nc = bacc.Bacc("TRN2", target_bir_lowering=False, debug=False)

    mem_d = nc.dram_tensor("mem", [BL, cfg.T, MEM], f32, kind="ExternalInput")
    hT_d = nc.dram_tensor("hT", [HID, BL], f32, kind="ExternalInput")
    whT_d = nc.dram_tensor("whT", [HID, ATTN], f32, kind="ExternalInput")
    wmT_d = nc.dram_tensor("wmT", [MEM, ATTN], bf16, kind="ExternalInput")
    vT_d = nc.dram_tensor("vT", [ATTN, 1], bf16, kind="ExternalInput")
    idx_d = nc.dram_tensor("idx", [128, BL, ICOL], i16, kind="ExternalInput")
    id_d = nc.dram_tensor("idm", [128, 128], bf16, kind="ExternalInput")
    kb_d = nc.dram_tensor("kb", [BL, NKEEP], f32, kind="ExternalInput")
    attn_o = nc.dram_tensor("attn_o", [BL, NKEEP], f32, kind="ExternalOutput")
    ctx_o = nc.dram_tensor("ctx_o", [BL, MEM], f32, kind="ExternalOutput")

    with tile.TileContext(nc) as tc:
        with tc.tile_pool(name="const", bufs=1) as const:
            wm_sb = const.tile([128, MC, ATTN], bf16)
            nc.sync.dma_start(wm_sb[:], wmT_d.rearrange("(mc p) a -> p mc a", p=128))
            vT_sb = const.tile([128, AC, 1], bf16)
            nc.sync.dma_start(vT_sb[:], vT_d.rearrange("(ac p) one -> p ac one", p=128))
            ident = const.tile([128, 128], bf16)
            nc.sync.dma_start(ident[:], id_d[:])
            ident1 = const.tile([1, 1], f32)
            nc.vector.memset(ident1[:], 1.0)
            idx_sb = const.tile([128, BL, ICOL], i16)
            nc.sync.dma_start(idx_sb[:], idx_d[:])
            q_sb = const.tile([128, AC, BL], f32)

            # ---- q = (w_h @ h) computed as q_T[a, b], contraction over HID
            with (
                tc.tile_pool(name="setup", bufs=1) as setup,
                tc.tile_pool(name="psum_q", bufs=1, space="PSUM") as psum_q,
            ):
                whT_sb = setup.tile([128, HC, ATTN], f32)
                nc.sync.dma_start(
                    whT_sb[:], whT_d.rearrange("(hc p) a -> p hc a", p=128)
                )
                hT_sb = setup.tile([128, HC, BL], f32)
                nc.sync.dma_start(hT_sb[:], hT_d.rearrange("(hc p) b -> p hc b", p=128))
                for a in range(AC):
                    q_ps = psum_q.tile([128, BL], f32)
                    for hc in range(HC):
                        nc.tensor.matmul(
                            q_ps[:],
                            whT_sb[:, hc, a * 128 : (a + 1) * 128],
                            hT_sb[:, hc, :],
                            start=(hc == 0),
                            stop=(hc == HC - 1),
                        )
                    nc.vector.tensor_copy(q_sb[:, a, :], q_ps[:])

            with (
                tc.tile_pool(name="g32", bufs=3) as g32p,
                tc.tile_pool(name="gbf", bufs=3) as gbfp,
                tc.tile_pool(name="mtp", bufs=2) as mtpp,
                tc.tile_pool(name="th", bufs=3) as thp,
                tc.tile_pool(name="sm", bufs=2) as smp,
                tc.tile_pool(name="esm", bufs=1) as esmp,
                tc.tile_pool(name="eseg", bufs=3) as esegp,
                tc.tile_pool(name="out", bufs=2) as outp,
                tc.tile_pool(name="ps_t", bufs=2, space="PSUM") as ps_t,
                tc.tile_pool(name="ps_k", bufs=2, space="PSUM") as ps_k,
                tc.tile_pool(name="ps_e", bufs=1, space="PSUM") as ps_e,
                tc.tile_pool(name="ps_a", bufs=1, space="PSUM") as ps_a,
                tc.tile_pool(name="ps_c", bufs=1, space="PSUM") as ps_c,
            ):
                # Softmax note: |e| <= ||v||_1 ~ 8, so exp() cannot overflow
                # and the classic max-subtraction is skipped entirely.  That
                # removes the all-of-e barrier: exp/transposes run per segment.
                CSEG = min(512, MEM)
                NMH = MEM // CSEG
                # even gather chunks of GCH rows
                chunks = [(i * GCH, GCH) for i in range(NKEEP // GCH)]
                nseg_total = sum(
                    (n + 511) // 512 for _, n in chunks
                )
                for b in range(BL):
                    kb_t = esmp.tile([1, NKEEP], f32, tag="kb")
                    nc.sync.dma_start(kb_t[:], kb_d[b : b + 1, :])
                    p_sb = smp.tile([1, NKEEP], f32, tag="p")
                    sc = smp.tile([1, 64], f32, tag="scalars")
                    ds = sc[:, 0:nseg_total]  # per-seg exp sums
                    rd = sc[:, 32:33]
                    atps = ps_a.tile([128, TC], f32, tag="atps")
                    at_bf = outp.tile([128, TC], bf16, tag="atbf")
                    cps = ps_c.tile([1, MEM], f32, tag="cps")

                    ev = 0
                    seg_i = 0
                    for co, cn in chunks:
                        GTCc = cn // 128
                        ns = (cn + 511) // 512
                        segs = [(i * (cn // ns), cn // ns) for i in range(ns)]
                        g32 = g32p.tile([128, GTCc, MEM], f32, tag="g32")
                        nc.gpsimd.dma_gather(
                            g32[:],
                            mem_d[b],
                            idx_sb[:, b, co // 16 : (co + cn) // 16],
                            num_idxs=cn,
                            num_idxs_reg=cn,
                            elem_size=MEM,
                        )
                        # cast to bf16, split fine for pipelining
                        gbf = gbfp.tile([128, GTCc, MEM], bf16, tag="gbf")
                        for g in range(GTCc):
                            nc.vector.tensor_copy(gbf[:, g, :], g32[:, g, :])
                        # transpose this chunk: [t', m] -> [m, t'] per 128-block
                        mtp = mtpp.tile([128, MC, cn], bf16, tag="mtp")
                        for mc in range(MC):
                            tps = ps_t.tile([128, cn], bf16, tag="tps")
                            for g in range(GTCc):
                                nc.tensor.transpose(
                                    tps[:, g * 128 : (g + 1) * 128],
                                    gbf[:, g, mc * 128 : (mc + 1) * 128],
                                    ident[:],
                                )
                            if ev % 4 != 3:
                                nc.vector.tensor_copy(mtp[:, mc, :], tps[:])
                            else:
                                nc.scalar.copy(mtp[:, mc, :], tps[:])
                            ev += 1
                        # k matmuls + tanh per (a-chunk, segment)
                        ths = []
                        for a in range(AC):
                            th_t = thp.tile([128, cn], bf16, tag="th")
                            ths.append(th_t)
                            for so, sn in segs:
                                kps = ps_k.tile([128, 512], f32, tag="kps")
                                for mc in range(MC):
                                    nc.tensor.matmul(
                                        kps[:, :sn],
                                        wm_sb[:, mc, a * 128 : (a + 1) * 128],
                                        mtp[:, mc, so : so + sn],
                                        start=(mc == 0),
                                        stop=(mc == MC - 1),
                                    )
                                nc.scalar.activation(
                                    out=th_t[:, so : so + sn],
                                    in_=kps[:, :sn],
                                    func=Act.Tanh,
                                    bias=q_sb[:, a, b : b + 1],
                                    scale=1.0,
                                )
                        # e matvec per segment; pad-bias folded into psum evac;
                        # exp runs per segment (no max barrier)
                        for so, sn in segs:
                            off2 = co + so
                            eps = ps_e.tile([1, 512], f32, tag="eps")
                            for a in range(AC):
                                nc.tensor.matmul(
                                    eps[:, :sn],
                                    vT_sb[:, a, :],
                                    ths[a][:, so : so + sn],
                                    start=(a == 0),
                                    stop=(a == AC - 1),
                                )
                            e_seg = esegp.tile([1, 512], f32, tag="eseg")
                            nc.vector.tensor_add(
                                e_seg[:, :sn], eps[:, :sn], kb_t[:, off2 : off2 + sn]
                            )
                            nc.scalar.activation(
                                out=p_sb[:, off2 : off2 + sn],
                                in_=e_seg[:, :sn],
                                func=Act.Exp,
                                accum_out=ds[:, seg_i : seg_i + 1],
                            )
                            seg_i += 1
                        # transpose p for this chunk and accumulate context
                        for g in range(GTCc):
                            tcc = co // 128 + g
                            nc.tensor.transpose(
                                atps[:, tcc : tcc + 1],
                                p_sb[:, tcc * 128 : (tcc + 1) * 128],
                                ident1[:],
                            )
                        nc.vector.tensor_copy(
                            at_bf[:, co // 128 : co // 128 + GTCc],
                            atps[:, co // 128 : co // 128 + GTCc],
                        )
                        for mh in range(NMH):
                            for g in range(GTCc):
                                tcc = co // 128 + g
                                nc.tensor.matmul(
                                    cps[:, mh * CSEG : (mh + 1) * CSEG],
                                    at_bf[:, tcc : tcc + 1],
                                    gbf[:, g, mh * CSEG : (mh + 1) * CSEG],
                                    start=(tcc == 0),
                                    stop=(tcc == TC - 1),
                                )

                    # ---- denominator + normalization (context scaled by 1/D
                    # during psum evac; attn normalized on the way out)
                    nc.vector.reduce_sum(rd, ds, axis=mybir.AxisListType.X)
                    nc.vector.reciprocal(rd, rd)
                    attn_n = esmp.tile([1, NKEEP], f32, tag="attn_n")
                    nc.scalar.activation(
                        out=attn_n[:], in_=p_sb[:], func=Act.Copy, scale=rd
                    )
                    nc.sync.dma_start(attn_o[b : b + 1, :], attn_n[:])
                    ctx_sb = outp.tile([1, MEM], f32, tag="ctx")
                    nc.vector.tensor_scalar_mul(ctx_sb[:], cps[:], rd)
                    nc.sync.dma_start(ctx_o[b : b + 1, :], ctx_sb[:])

    nc.compile()
    return nc


def host_prep(cfg: Cfg, memory_mask_local: np.ndarray):
    """Per-core index/bias prep. memory_mask_local: [BL, T] bool.

    Returns idx [128, BL, NKEEP//16] i16, kb [BL, NKEEP] f32, and the raw
    per-batch kept index lists (for host-side scatter of outputs).
    """
    BL, NKEEP = cfg.BL, cfg.NKEEP
    idx = np.zeros((128, BL, NKEEP // 16), dtype=np.int16)
    kb = np.zeros((BL, NKEEP), dtype=np.float32)
    kept_lists = []
    for b in range(BL):
        kept = np.nonzero(memory_mask_local[b])[0]
        cnt = len(kept)
        assert cnt <= NKEEP, f"kept count {cnt} exceeds NKEEP={NKEEP}"
        kept_lists.append(kept)
        arr = np.zeros(NKEEP, dtype=np.int16)
        arr[:cnt] = kept.astype(np.int16)
        kb[b, cnt:] = -1e30
        wrapped = arr.reshape(NKEEP // 16, 16).T  # [16, cols]
        idx[:, b, :] = np.tile(wrapped, (8, 1))
    return idx, kb, kept_lists


_CACHE = {}

# Test knobs (harness-invisible): TRACE captures an NTFF profile on the run;
# LAST_RESULT holds the most recent BassKernelResults.
TRACE = False
LAST_RESULT = None


def _get_module():
    if "nc" not in _CACHE:
        _CACHE["nc"] = build(FULL)
    return _CACHE["nc"]


def kernel(h_t, memory, memory_mask, w_h, w_m, v):
    import ml_dtypes

    cfg = FULL
    B, T, MEM = memory.shape
    n_cores = 8
    BL = B // n_cores
    assert BL == cfg.BL and T == cfg.T and MEM == cfg.MEM

    h_t = np.asarray(h_t, dtype=np.float32)
    memory = np.asarray(memory, dtype=np.float32)
    memory_mask = np.asarray(memory_mask)
    w_h = np.asarray(w_h, dtype=np.float32)
    w_m = np.asarray(w_m, dtype=np.float32)
    v = np.asarray(v, dtype=np.float32)

    whT = np.ascontiguousarray(w_h.T)  # [HID, ATTN] f32
    wmT = np.ascontiguousarray(w_m.T).astype(ml_dtypes.bfloat16)  # [MEM, ATTN]
    vT = np.ascontiguousarray(v.reshape(1, -1).T).astype(ml_dtypes.bfloat16)

    idm = np.eye(128, dtype=ml_dtypes.bfloat16)
    in_maps = []
    kept_all = []
    for c in range(n_cores):
        sl = slice(c * BL, (c + 1) * BL)
        idx, kb, kept_lists = host_prep(cfg, memory_mask[sl])
        kept_all.append(kept_lists)
        in_maps.append(
            {
                "mem": np.ascontiguousarray(memory[sl]),
                "hT": np.ascontiguousarray(h_t[sl].T),
                "whT": whT,
                "wmT": wmT,
                "vT": vT,
                "idx": idx,
                "kb": kb,
                "idm": idm,
            }
        )

    from concourse.bass_utils import run_bass_kernel_spmd

    nc = _get_module()
    res = run_bass_kernel_spmd(nc, in_maps, core_ids=list(range(n_cores)), trace=TRACE)
    globals()["LAST_RESULT"] = res

    context = np.zeros((B, MEM), dtype=np.float32)
    attn = np.zeros((B, T), dtype=np.float32)
    for c in range(n_cores):
        r = res.results[c]
        for b in range(cfg.BL):
            g = c * cfg.BL + b
            kept = kept_all[c][b]
            attn[g, kept] = r["attn_o"][b, : len(kept)]
            context[g] = r["ctx_o"][b]
    return context, attn


# revision 28
# speedup vs baseline: 1.1638x; 1.0288x over previous
"""AdditiveAttention (Bahdanau) kernel for Trainium2, 8-way data-parallel over batch.

Math (per batch row b):
    q = h_t @ w_h.T                       [ATTN]
    k[t] = memory[t] @ w_m.T              [T, ATTN]
    e[t] = v . tanh(q + k[t])             [T]
    attn = softmax(mask ? e : -inf)       [T]
    context = attn @ memory               [MEM]

Strategy: the boolean mask kills ~half the rows exactly (attn == 0 there), so
the device only ever touches the unmasked rows.  The host computes, per batch,
the sorted list of kept row indices (padded to NKEEP with index 0 and an
additive -1e30 bias so padded slots vanish in the softmax), and the kernel
gathers exactly those rows with dma_gather.  Everything else is a single
fused pass per batch: gather -> bf16 cast -> PE transpose (for the k matmul,
which contracts over MEM) -> k matmul -> tanh(+q) -> e matvec -> masked
softmax -> attn transpose -> context matmul.  Context/attn are scattered back
to dense [B, T] on the host.
"""

import math
from dataclasses import dataclass

import numpy as np


@dataclass(frozen=True)
class Cfg:
    BL: int = 8  # batches per core
    T: int = 4096
    MEM: int = 1024
    HID: int = 1024
    ATTN: int = 256
    NKEEP: int = 2304  # 18*128; P(Binom(4096,.5) > 2304) ~ 6e-16
    GCH: int = 768  # gather chunk (rows per dma_gather), multiple of 128

    @property
    def MC(self):
        return self.MEM // 128

    @property
    def AC(self):
        return self.ATTN // 128

    @property
    def HC(self):
        return self.HID // 128

    @property
    def TSEG(self):
        return self.GCH // 2  # matmul moving-dim segment, <=512

    @property
    def NCH(self):
        return self.NKEEP // self.GCH  # gather chunks per batch

    @property
    def TC(self):
        return self.NKEEP // 128  # 128-row chunks per batch


FULL = Cfg()


def build(cfg: Cfg):
    import concourse.mybir as mybir
    import concourse.tile as tile
    from concourse import bacc

    f32 = mybir.dt.float32
    f32r = mybir.dt.float32r
    bf16 = mybir.dt.bfloat16
    i16 = mybir.dt.int16
    Act = mybir.ActivationFunctionType

    BL, MEM, HID, ATTN, NKEEP, GCH = (
        cfg.BL,
        cfg.MEM,
        cfg.HID,
        cfg.ATTN,
        cfg.NKEEP,
        cfg.GCH,
    )
    MC, AC, HC, TSEG, NCH, TC = cfg.MC, cfg.AC, cfg.HC, cfg.TSEG, cfg.NCH, cfg.TC
    ICOL = NKEEP // 16  # idx columns total
    ICH = GCH // 16  # idx columns per gather chunk
    GTC = GCH // 128  # 128-row chunks per gather chunk

    nc = bacc.Bacc("TRN2", target_bir_lowering=False, debug=False)

    mem_d = nc.dram_tensor("mem", [BL, cfg.T, MEM], f32, kind="ExternalInput")
    hT_d = nc.dram_tensor("hT", [HID, BL], f32, kind="ExternalInput")
    whT_d = nc.dram_tensor("whT", [HID, ATTN], f32, kind="ExternalInput")
    wmT_d = nc.dram_tensor("wmT", [MEM, ATTN], bf16, kind="ExternalInput")
    vT_d = nc.dram_tensor("vT", [ATTN, 1], f32r, kind="ExternalInput")
    idx_d = nc.dram_tensor("idx", [128, BL, ICOL], i16, kind="ExternalInput")
    id_d = nc.dram_tensor("idm", [128, 128], bf16, kind="ExternalInput")
    kb_d = nc.dram_tensor("kb", [BL, NKEEP], f32, kind="ExternalInput")
    attn_o = nc.dram_tensor("attn_o", [BL, NKEEP], f32, kind="ExternalOutput")
    ctx_o = nc.dram_tensor("ctx_o", [BL, MEM], f32, kind="ExternalOutput")

    with tile.TileContext(nc) as tc:
        with tc.tile_pool(name="const", bufs=1) as const:
            wm_sb = const.tile([128, MC, ATTN], bf16)
            nc.sync.dma_start(wm_sb[:], wmT_d.rearrange("(mc p) a -> p mc a", p=128))
            vT_sb = const.tile([128, AC, 1], f32r)
            nc.sync.dma_start(vT_sb[:], vT_d.rearrange("(ac p) one -> p ac one", p=128))
            ident = const.tile([128, 128], bf16)
            nc.sync.dma_start(ident[:], id_d[:])
            ident1 = const.tile([1, 1], f32)
            nc.vector.memset(ident1[:], 1.0)
            idx_sb = const.tile([128, BL, ICOL], i16)
            nc.sync.dma_start(idx_sb[:], idx_d[:])
            q_sb = const.tile([128, AC, BL], f32)

            # ---- q = (w_h @ h) computed as q_T[a, b], contraction over HID
            with (
                tc.tile_pool(name="setup", bufs=1) as setup,
                tc.tile_pool(name="psum_q", bufs=1, space="PSUM") as psum_q,
            ):
                whT_sb = setup.tile([128, HC, ATTN], f32)
                nc.sync.dma_start(
                    whT_sb[:], whT_d.rearrange("(hc p) a -> p hc a", p=128)
                )
                hT_sb = setup.tile([128, HC, BL], f32)
                nc.sync.dma_start(hT_sb[:], hT_d.rearrange("(hc p) b -> p hc b", p=128))
                for a in range(AC):
                    q_ps = psum_q.tile([128, BL], f32)
                    for hc in range(HC):
                        nc.tensor.matmul(
                            q_ps[:],
                            whT_sb[:, hc, a * 128 : (a + 1) * 128],
                            hT_sb[:, hc, :],
                            start=(hc == 0),
                            stop=(hc == HC - 1),
                        )
                    nc.vector.tensor_copy(q_sb[:, a, :], q_ps[:])

            with (
                tc.tile_pool(name="g32", bufs=3) as g32p,
                tc.tile_pool(name="gbf", bufs=3) as gbfp,
                tc.tile_pool(name="mtp", bufs=2) as mtpp,
                tc.tile_pool(name="th", bufs=3) as thp,
                tc.tile_pool(name="sm", bufs=2) as smp,
                tc.tile_pool(name="esm", bufs=1) as esmp,
                tc.tile_pool(name="eseg", bufs=3) as esegp,
                tc.tile_pool(name="out", bufs=2) as outp,
                tc.tile_pool(name="ps_t", bufs=2, space="PSUM") as ps_t,
                tc.tile_pool(name="ps_k", bufs=2, space="PSUM") as ps_k,
                tc.tile_pool(name="ps_e", bufs=1, space="PSUM") as ps_e,
                tc.tile_pool(name="ps_a", bufs=1, space="PSUM") as ps_a,
                tc.tile_pool(name="ps_c", bufs=1, space="PSUM") as ps_c,
            ):
                # Softmax note: |e| <= ||v||_1 ~ 8, so exp() cannot overflow
                # and the classic max-subtraction is skipped entirely.  That
                # removes the all-of-e barrier: exp/transposes run per segment.
                CSEG = min(512, MEM)
                NMH = MEM // CSEG
                # even gather chunks of GCH rows
                chunks = [(i * GCH, GCH) for i in range(NKEEP // GCH)]
                nseg_total = sum(
                    (n + 511) // 512 for _, n in chunks
                )
                for b in range(BL):
                    kb_t = esmp.tile([1, NKEEP], f32, tag="kb")
                    nc.sync.dma_start(kb_t[:], kb_d[b : b + 1, :])
                    p_sb = smp.tile([1, NKEEP], f32, tag="p")
                    sc = smp.tile([1, 64], f32, tag="scalars")
                    ds = sc[:, 0:nseg_total]  # per-seg exp sums
                    rd = sc[:, 32:33]
                    atps = ps_a.tile([128, TC], f32, tag="atps")
                    at_bf = outp.tile([128, TC], bf16, tag="atbf")
                    cps = ps_c.tile([1, MEM], f32, tag="cps")

                    ev = 0
                    seg_i = 0
                    for co, cn in chunks:
                        GTCc = cn // 128
                        ns = (cn + 511) // 512
                        segs = [(i * (cn // ns), cn // ns) for i in range(ns)]
                        g32 = g32p.tile([128, GTCc, MEM], f32, tag="g32")
                        nc.gpsimd.dma_gather(
                            g32[:],
                            mem_d[b],
                            idx_sb[:, b, co // 16 : (co + cn) // 16],
                            num_idxs=cn,
                            num_idxs_reg=cn,
                            elem_size=MEM,
                        )
                        # cast to bf16, split fine for pipelining
                        gbf = gbfp.tile([128, GTCc, MEM], bf16, tag="gbf")
                        for g in range(GTCc):
                            nc.vector.tensor_copy(gbf[:, g, :], g32[:, g, :])
                        # transpose this chunk: [t', m] -> [m, t'] per 128-block
                        mtp = mtpp.tile([128, MC, cn], bf16, tag="mtp")
                        for mc in range(MC):
                            tps = ps_t.tile([128, cn], bf16, tag="tps")
                            for g in range(GTCc):
                                nc.tensor.transpose(
                                    tps[:, g * 128 : (g + 1) * 128],
                                    gbf[:, g, mc * 128 : (mc + 1) * 128],
                                    ident[:],
                                )
                            if ev % 4 != 3:
                                nc.vector.tensor_copy(mtp[:, mc, :], tps[:])
                            else:
                                nc.scalar.copy(mtp[:, mc, :], tps[:])
                            ev += 1
                        # k matmuls + tanh per (a-chunk, segment)
                        ths = []
                        for a in range(AC):
                            th_t = thp.tile([128, cn], f32r, tag="th")
                            ths.append(th_t)
                            for so, sn in segs:
                                kps = ps_k.tile([128, 512], f32, tag="kps")
                                for mc in range(MC):
                                    nc.tensor.matmul(
                                        kps[:, :sn],
                                        wm_sb[:, mc, a * 128 : (a + 1) * 128],
                                        mtp[:, mc, so : so + sn],
                                        start=(mc == 0),
                                        stop=(mc == MC - 1),
                                    )
                                nc.scalar.activation(
                                    out=th_t[:, so : so + sn],
                                    in_=kps[:, :sn],
                                    func=Act.Tanh,
                                    bias=q_sb[:, a, b : b + 1],
                                    scale=1.0,
                                )
                        # e matvec per segment; pad-bias folded into psum evac;
                        # exp runs per segment (no max barrier)
                        for so, sn in segs:
                            off2 = co + so
                            eps = ps_e.tile([1, 512], f32, tag="eps")
                            for a in range(AC):
                                nc.tensor.matmul(
                                    eps[:, :sn],
                                    vT_sb[:, a, :],
                                    ths[a][:, so : so + sn],
                                    start=(a == 0),
                                    stop=(a == AC - 1),
                                )
                            e_seg = esegp.tile([1, 512], f32, tag="eseg")
                            nc.vector.tensor_add(
                                e_seg[:, :sn], eps[:, :sn], kb_t[:, off2 : off2 + sn]
                            )
                            nc.scalar.activation(
                                out=p_sb[:, off2 : off2 + sn],
                                in_=e_seg[:, :sn],
                                func=Act.Exp,
                                accum_out=ds[:, seg_i : seg_i + 1],
                            )
                            seg_i += 1
                        # transpose p for this chunk and accumulate context
                        for g in range(GTCc):
                            tcc = co // 128 + g
                            nc.tensor.transpose(
                                atps[:, tcc : tcc + 1],
                                p_sb[:, tcc * 128 : (tcc + 1) * 128],
                                ident1[:],
                            )
                        nc.vector.tensor_copy(
                            at_bf[:, co // 128 : co // 128 + GTCc],
                            atps[:, co // 128 : co // 128 + GTCc],
                        )
                        for mh in range(NMH):
                            for g in range(GTCc):
                                tcc = co // 128 + g
                                nc.tensor.matmul(
                                    cps[:, mh * CSEG : (mh + 1) * CSEG],
                                    at_bf[:, tcc : tcc + 1],
                                    gbf[:, g, mh * CSEG : (mh + 1) * CSEG],
                                    start=(tcc == 0),
                                    stop=(tcc == TC - 1),
                                )

                    # ---- denominator + normalization (context scaled by 1/D
                    # during psum evac; attn normalized on the way out)
                    nc.vector.reduce_sum(rd, ds, axis=mybir.AxisListType.X)
                    nc.vector.reciprocal(rd, rd)
                    attn_n = esmp.tile([1, NKEEP], f32, tag="attn_n")
                    nc.scalar.activation(
                        out=attn_n[:], in_=p_sb[:], func=Act.Copy, scale=rd
                    )
                    nc.sync.dma_start(attn_o[b : b + 1, :], attn_n[:])
                    ctx_sb = outp.tile([1, MEM], f32, tag="ctx")
                    nc.vector.tensor_scalar_mul(ctx_sb[:], cps[:], rd)
                    nc.sync.dma_start(ctx_o[b : b + 1, :], ctx_sb[:])

    nc.compile()
    return nc


def host_prep(cfg: Cfg, memory_mask_local: np.ndarray):
    """Per-core index/bias prep. memory_mask_local: [BL, T] bool.

    Returns idx [128, BL, NKEEP//16] i16, kb [BL, NKEEP] f32, and the raw
    per-batch kept index lists (for host-side scatter of outputs).
    """
    BL, NKEEP = cfg.BL, cfg.NKEEP
    idx = np.zeros((128, BL, NKEEP // 16), dtype=np.int16)
    kb = np.zeros((BL, NKEEP), dtype=np.float32)
    kept_lists = []
    for b in range(BL):
        kept = np.nonzero(memory_mask_local[b])[0]
        cnt = len(kept)
        assert cnt <= NKEEP, f"kept count {cnt} exceeds NKEEP={NKEEP}"
        kept_lists.append(kept)
        arr = np.zeros(NKEEP, dtype=np.int16)
        arr[:cnt] = kept.astype(np.int16)
        kb[b, cnt:] = -1e30
        wrapped = arr.reshape(NKEEP // 16, 16).T  # [16, cols]
        idx[:, b, :] = np.tile(wrapped, (8, 1))
    return idx, kb, kept_lists


_CACHE = {}

# Test knobs (harness-invisible): TRACE captures an NTFF profile on the run;
# LAST_RESULT holds the most recent BassKernelResults.
TRACE = False
LAST_RESULT = None


def _get_module():
    if "nc" not in _CACHE:
        _CACHE["nc"] = build(FULL)
    return _CACHE["nc"]


def kernel(h_t, memory, memory_mask, w_h, w_m, v):
    import ml_dtypes

    cfg = FULL
    B, T, MEM = memory.shape
    n_cores = 8
    BL = B // n_cores
    assert BL == cfg.BL and T == cfg.T and MEM == cfg.MEM

    h_t = np.asarray(h_t, dtype=np.float32)
    memory = np.asarray(memory, dtype=np.float32)
    memory_mask = np.asarray(memory_mask)
    w_h = np.asarray(w_h, dtype=np.float32)
    w_m = np.asarray(w_m, dtype=np.float32)
    v = np.asarray(v, dtype=np.float32)

    whT = np.ascontiguousarray(w_h.T)  # [HID, ATTN] f32
    wmT = np.ascontiguousarray(w_m.T).astype(ml_dtypes.bfloat16)  # [MEM, ATTN]
    vT = np.ascontiguousarray(v.reshape(1, -1).T).astype(np.float32)

    idm = np.eye(128, dtype=ml_dtypes.bfloat16)
    in_maps = []
    kept_all = []
    for c in range(n_cores):
        sl = slice(c * BL, (c + 1) * BL)
        idx, kb, kept_lists = host_prep(cfg, memory_mask[sl])
        kept_all.append(kept_lists)
        in_maps.append(
            {
                "mem": np.ascontiguousarray(memory[sl]),
                "hT": np.ascontiguousarray(h_t[sl].T),
                "whT": whT,
                "wmT": wmT,
                "vT": vT,
                "idx": idx,
                "kb": kb,
                "idm": idm,
            }
        )

    from concourse.bass_utils import run_bass_kernel_spmd

    nc = _get_module()
    res = run_bass_kernel_spmd(nc, in_maps, core_ids=list(range(n_cores)), trace=TRACE)
    globals()["LAST_RESULT"] = res

    context = np.zeros((B, MEM), dtype=np.float32)
    attn = np.zeros((B, T), dtype=np.float32)
    for c in range(n_cores):
        r = res.results[c]
        for b in range(cfg.BL):
            g = c * cfg.BL + b
            kept = kept_all[c][b]
            attn[g, kept] = r["attn_o"][b, : len(kept)]
            context[g] = r["ctx_o"][b]
    return context, attn
